# revision 1
# baseline (speedup 1.0000x reference)
# CenterNet GT-heatmap kernel for Trainium2 (Bass/Tile), 8-core SPMD.
#
# Math: out[n,0,y,x] = max_o gy[o,y]*gx[o,x], where gx/gy are 1-D Gaussian
# profiles clipped to |d| <= r (separable form of the umich gaussian patch).
# Objects are partitioned per image into "slots" (pairwise rect-disjoint
# sets); within a slot, max == sum, so TensorE computes each slot with one
# K=32 matmul (sum of rank-1 outer products) into its own PSUM block, and
# VectorE max-reduces the blocks.
#
# Slot layout is uniform across images/cores (SPMD: one program). Per image:
#   slot (b, j) for base b in {0,1,2,3} x phase j in {0,1}, plus slot (0,2).
#   Slot (b,j) objects live in rows [32b,32b+32) of the image's main profile
#   tile (phase mask selects rows per j) and optionally rows [32b,32b+32) of
#   a shared overflow tile (image A owns ovf bases 0,1; image B bases 2,3),
#   whose matmul accumulates (start=False) into the same PSUM block.
# PSUM rule respected by construction: each physical PSUM bank only ever
# receives matmuls with one operand base partition (bank 32b <- base 32b);
# mixing bases within a bank hangs the device (found empirically).

import numpy as np

N_IMG, C_CH, H, W, NOBJ, N_CORES = 16, 1, 128, 128, 128, 8
IMGS_PER_CORE = N_IMG // N_CORES  # 2

_PROG_CACHE = {}


# ---------------------------------------------------------------- host math
def _gaussian_radius_f32(w, h):
    mo = np.float32(0.7)
    one = np.float32(1.0)
    b1 = h + w
    c1 = w * h * (one - mo) / (one + mo)
    sq1 = np.sqrt(b1 * b1 - np.float32(4.0) * np.float32(1.0) * c1)
    r1 = (b1 + sq1) / np.float32(2.0)
    b2 = np.float32(2.0) * (h + w)
    c2 = (one - mo) * w * h
    sq2 = np.sqrt(b2 * b2 - np.float32(4.0) * np.float32(4.0) * c2)
    r2 = (b2 + sq2) / np.float32(2.0)
    a3 = np.float32(4.0) * mo
    b3 = np.float32(-2.0) * mo * (h + w)
    c3 = (mo - one) * w * h
    sq3 = np.sqrt(b3 * b3 - np.float32(4.0) * a3 * c3)
    r3 = (b3 + sq3) / np.float32(2.0)
    return np.minimum(np.minimum(r1, r2), r3)


def _object_params(bboxes):
    """Replicate reference fp32 math; per-object ints + gaussian coefs."""
    b = bboxes.astype(np.float32)
    x1 = b[..., 0] * np.float32(W)
    y1 = b[..., 1] * np.float32(H)
    x2 = b[..., 2] * np.float32(W)
    y2 = b[..., 3] * np.float32(H)
    bw = x2 - x1
    bh = y2 - y1
    with np.errstate(invalid="ignore"):
        radius = _gaussian_radius_f32(bw, bh)
    radius = np.maximum(np.nan_to_num(radius), np.float32(0.0))
    cx = (x1 + x2) * np.float32(0.5)
    cy = (y1 + y2) * np.float32(0.5)
    cxi = cx.astype(np.int32)
    cyi = cy.astype(np.int32)
    ri = radius.astype(np.int32)
    valid = (b[..., 4] == 1) & (bw > 0) & (bh > 0)
    sigma = (2 * ri + 1).astype(np.float32) / np.float32(6.0)

    # device computes s = (x*sqk + bsx)^2 ~= k*(x-cx)^2; mask s <= t.
    D = np.float32(2.0) * sigma * sigma  # 2*sigma^2, fp32 like reference
    k64 = 1.0 / D.astype(np.float64)
    sqk = np.sqrt(k64).astype(np.float32)
    sqk64 = sqk.astype(np.float64)
    bsx = (-sqk64 * cxi).astype(np.float32)
    bsy = (-sqk64 * cyi).astype(np.float32)
    rif = ri.astype(np.float64)
    t = (sqk64 * sqk64 * (rif * rif + rif + 0.5)).astype(np.float32)
    return cxi, cyi, ri, valid, sqk, bsx, bsy, t


# ------------------------------------------------------------- slot fitting
def _fit_image(cxi, cyi, ri, valid, j2_all):
    """Assign objects of one image to slots (b,j). Returns None if infeasible,
    else list of (obj, b, j, pool) with pool in {'main','ovf'}."""
    x0 = np.maximum(cxi - ri, 0)
    x1 = np.minimum(cxi + ri, W - 1)
    y0 = np.maximum(cyi - ri, 0)
    y1 = np.minimum(cyi + ri, H - 1)
    nobj = len(cxi)
    conf = np.zeros((nobj, nobj), bool)
    for a in range(nobj):
        conf[a] = (
            (np.maximum(x0[a], x0) <= np.minimum(x1[a], x1))
            & (np.maximum(y0[a], y0) <= np.minimum(y1[a], y1))
        )
    np.fill_diagonal(conf, False)
    order = np.argsort(-conf.sum(1), kind="stable")

    j2_bases = [0, 1, 2, 3] if j2_all else [0]
    slot_order = [(b, j) for j in (0, 1) for b in range(4)] + [
        (b, 2) for b in j2_bases
    ]
    members = {s: [] for s in slot_order}
    main_used = [0, 0, 0, 0]
    ovf_used = [0, 0, 0, 0]
    # ovf capacity per base is decided by the caller via ovf_cap
    placements = []
    for o in order:
        if not valid[o]:
            continue
        done = False
        for (b, j) in slot_order:
            if main_used[b] >= 32 and (j == 2 or ovf_used[b] >= 32):
                continue
            if any(conf[o, p] for p in members[(b, j)]):
                continue
            members[(b, j)].append(o)
            if main_used[b] < 32:
                main_used[b] += 1
                pool = "main"
            else:
                if j == 2:
                    members[(b, j)].pop()
                    continue
                ovf_used[b] += 1
                pool = "ovf"
            placements.append((int(o), b, j, pool))
            done = True
            break
        if not done:
            return None
    return placements


def _fit_image_for_core(cxi, cyi, ri, valid, img_pos, j2_all):
    """img_pos 0 => image A (ovf bases 0,1); 1 => image B (ovf bases 2,3).
    Wraps _fit_image but zeroes ovf capacity on the other image's bases by
    retrying with a mask."""
    # monkey-ish: re-run fit with allowed ovf bases
    allowed = (0, 1) if img_pos == 0 else (2, 3)
    x0 = np.maximum(cxi - ri, 0)
    x1 = np.minimum(cxi + ri, W - 1)
    y0 = np.maximum(cyi - ri, 0)
    y1 = np.minimum(cyi + ri, H - 1)
    nobj = len(cxi)
    conf = np.zeros((nobj, nobj), bool)
    for a in range(nobj):
        conf[a] = (
            (np.maximum(x0[a], x0) <= np.minimum(x1[a], x1))
            & (np.maximum(y0[a], y0) <= np.minimum(y1[a], y1))
        )
    np.fill_diagonal(conf, False)
    order = np.argsort(-conf.sum(1), kind="stable")

    j2_bases = [0, 1, 2, 3] if j2_all else [0]
    slot_order = [(b, j) for j in (0, 1) for b in range(4)] + [
        (b, 2) for b in j2_bases
    ]
    members = {s: [] for s in slot_order}
    main_used = [0, 0, 0, 0]
    ovf_used = [0, 0, 0, 0]
    ovf_cap = [32 if b in allowed else 0 for b in range(4)]
    placements = []
    for o in order:
        if not valid[o]:
            continue
        done = False
        for (b, j) in slot_order:
            has_room = main_used[b] < 32 or (j != 2 and ovf_used[b] < ovf_cap[b])
            if not has_room:
                continue
            if any(conf[o, p] for p in members[(b, j)]):
                continue
            members[(b, j)].append(o)
            if main_used[b] < 32:
                main_used[b] += 1
                pool = "main"
            else:
                ovf_used[b] += 1
                pool = "ovf"
            placements.append((int(o), b, j, pool))
            done = True
            break
        if not done:
            return None
    return placements


# ------------------------------------------------------------- program build
def _build_program(j2_all):
    import concourse.bass as bass
    import concourse.bacc as bacc
    import concourse.tile as tile
    from concourse import mybir

    f32 = mybir.dt.float32
    AF = mybir.ActivationFunctionType
    ALU = mybir.AluOpType

    n_phase_main = 3  # j0, j1, j2
    j2_bases = [0, 1, 2, 3] if j2_all else [0]

    nc = bacc.Bacc("TRN2", target_bir_lowering=False, debug=False,
                   num_devices=N_CORES)
    xsrow = nc.dram_tensor("xsrow", [1, W], f32, kind="ExternalInput")
    # coef columns: MA 0-6 (sqk,bsx,bsy,t,m0,m1,m2), MB 7-13, OV 14-19
    coef = nc.dram_tensor("coef", [128, 20], f32, kind="ExternalInput")
    out = nc.dram_tensor("out", [IMGS_PER_CORE, C_CH, H, W], f32,
                         kind="ExternalOutput")

    with tile.TileContext(nc) as tc:
        with tc.tile_pool(name="sb", bufs=1) as sb, \
             tc.tile_pool(name="ps", bufs=1, space="PSUM") as psp:
            XS = sb.tile([128, W], f32, tag="xs")
            src = xsrow.ap()
            bcast = bass.AP(tensor=src.tensor, offset=src.offset,
                            ap=[[0, 128]] + [src.ap[-1]])
            nc.sync.dma_start(XS[:], bcast)
            CF = sb.tile([128, 20], f32, tag="cf")
            nc.sync.dma_start(CF[:], coef.ap()[:, :])

            # profile sets: (name, coef col base, n phases)
            sets = [("MA", 0, n_phase_main), ("MB", 7, n_phase_main),
                    ("OV", 14, 2)]
            gx = {}
            gy = {}
            for name, cb, nph in sets:
                s = sb.tile([128, 2 * W], f32, tag=f"s{name}")
                nc.scalar.activation(s[:, 0:W], XS[:], AF.Square,
                                     bias=CF[:, cb + 1:cb + 2],
                                     scale=CF[:, cb:cb + 1])
                nc.scalar.activation(s[:, W:2 * W], XS[:], AF.Square,
                                     bias=CF[:, cb + 2:cb + 3],
                                     scale=CF[:, cb:cb + 1])
                e = sb.tile([128, 2 * W], f32, tag=f"e{name}")
                nc.scalar.activation(e[:], s[:], AF.Exp, bias=0.0, scale=-1.0)
                gxt = sb.tile([128, W], f32, tag=f"gx{name}")
                nc.vector.scalar_tensor_tensor(
                    gxt[:], s[:, 0:W], CF[:, cb + 3:cb + 4], e[:, 0:W],
                    ALU.is_le, ALU.mult)
                gyall = sb.tile([128, W], f32, tag=f"gya{name}")
                nc.vector.scalar_tensor_tensor(
                    gyall[:], s[:, W:2 * W], CF[:, cb + 3:cb + 4],
                    e[:, W:2 * W], ALU.is_le, ALU.mult)
                gx[name] = gxt
                gy[name] = {}
                for j in range(nph):
                    gyj = sb.tile([128, W], f32, tag=f"gy{name}{j}")
                    nc.vector.tensor_scalar(
                        gyj[:], gyall[:], CF[:, cb + 4 + j:cb + 5 + j], None,
                        ALU.mult)
                    gy[name][j] = gyj

            for i in range(IMGS_PER_CORE):
                mn = "MA" if i == 0 else "MB"
                ovf_bases = (0, 1) if i == 0 else (2, 3)
                ps = psp.tile([128, 2048], f32, tag=f"ps{i}")
                for b in range(4):
                    base = 32 * b
                    has_ovf = b in ovf_bases
                    for j in (0, 1):
                        blk = ps[:, 512 * b + 128 * j:512 * b + 128 * j + 128]
                        nc.tensor.matmul(
                            blk, gy[mn][j][base:base + 32, :],
                            gx[mn][base:base + 32, :],
                            start=True, stop=not has_ovf,
                            tile_position=(base, 0))
                        if has_ovf:
                            nc.tensor.matmul(
                                blk, gy["OV"][j][base:base + 32, :],
                                gx["OV"][base:base + 32, :],
                                start=False, stop=True,
                                tile_position=(base, 0))
                # j2 blocks (main rows only)
                for b in j2_bases:
                    base = 32 * b
                    blk = ps[:, 512 * b + 256:512 * b + 256 + 128]
                    nc.tensor.matmul(
                        blk, gy[mn][2][base:base + 32, :],
                        gx[mn][base:base + 32, :],
                        start=True, stop=True, tile_position=(base, 0))

                red = sb.tile([128, W], f32, tag=f"red{i}")
                psap = ps[:]
                if j2_all:
                    ap3 = bass.AP(tensor=psap.tensor, offset=psap.offset,
                                  ap=[psap.ap[0], [1, 128], [512, 4], [128, 3]])
                    nc.vector.tensor_reduce(red[:], ap3,
                                            axis=mybir.AxisListType.XY,
                                            op=ALU.max)
                else:
                    ap3 = bass.AP(tensor=psap.tensor, offset=psap.offset,
                                  ap=[psap.ap[0], [1, 128], [512, 4], [128, 2]])
                    nc.vector.tensor_reduce(red[:], ap3,
                                            axis=mybir.AxisListType.XY,
                                            op=ALU.max)
                    nc.vector.tensor_max(red[:], red[:], ps[:, 256:384])
                nc.sync.dma_start(out.ap()[i, 0], red[:])

    nc.compile()
    return nc


# ------------------------------------------------------------------ kernel
def kernel(hm, bboxes):
    from concourse.bass_utils import run_bass_kernel_spmd

    assert hm.shape == (N_IMG, C_CH, H, W)
    assert bboxes.shape == (N_IMG, NOBJ, 5)
    cxi, cyi, ri, valid, sqk, bsx, bsy, t = _object_params(np.asarray(bboxes))

    # choose uniform layout: smallest program that fits every image
    for j2_all in (False, True):
        placements = []
        ok = True
        for i in range(N_IMG):
            p = _fit_image_for_core(cxi[i], cyi[i], ri[i], valid[i],
                                    i % IMGS_PER_CORE, j2_all)
            if p is None:
                ok = False
                break
            placements.append(p)
        if ok:
            break
    if not ok:
        raise RuntimeError("slot fitting failed for all layouts")

    key = bool(j2_all)
    if key not in _PROG_CACHE:
        _PROG_CACHE[key] = _build_program(j2_all)
    nc = _PROG_CACHE[key]

    xs = np.arange(W, dtype=np.float32)[None, :]
    in_maps = []
    for c in range(N_CORES):
        cf = np.zeros((128, 20), np.float32)
        cf[:, 3] = -1.0   # t defaults: mask everything (dummy rows)
        cf[:, 10] = -1.0
        cf[:, 17] = -1.0
        for pos in range(IMGS_PER_CORE):
            img = IMGS_PER_CORE * c + pos
            cb = 0 if pos == 0 else 7
            main_next = [0, 0, 0, 0]
            ovf_next = [0, 0, 0, 0]
            for (o, b, j, pool) in placements[img]:
                if pool == "main":
                    row = 32 * b + main_next[b]
                    main_next[b] += 1
                    col0 = cb
                else:
                    row = 32 * b + ovf_next[b]
                    ovf_next[b] += 1
                    col0 = 14
                cf[row, col0 + 0] = sqk[img, o]
                cf[row, col0 + 1] = bsx[img, o]
                cf[row, col0 + 2] = bsy[img, o]
                cf[row, col0 + 3] = t[img, o]
                cf[row, col0 + 4 + j] = 1.0
        in_maps.append({"xsrow": xs, "coef": cf})

    res = run_bass_kernel_spmd(nc, in_maps, core_ids=list(range(N_CORES)))
    out = np.concatenate([res.results[c]["out"] for c in range(N_CORES)], 0)
    return out.astype(np.float32)


# revision 11
# speedup vs baseline: 1.0583x; 1.0583x over previous
# CenterNet GT-heatmap kernel for Trainium2 (Bass/Tile), 8-core SPMD.
#
# Math: out[n,0,y,x] = max_o gy[o,y]*gx[o,x], where gx/gy are 1-D Gaussian
# profiles clipped to |d| <= r (separable form of the umich gaussian patch).
# Objects are partitioned per image into "slots" (pairwise rect-disjoint
# sets); within a slot, max == sum, so TensorE computes slots as K=32
# matmuls (sums of rank-1 outer products) into PSUM blocks, and VectorE
# max-reduces the blocks. Phase-packing: one matmul per base covers all
# phases via a phase-masked, concatenated rhs (N = nph*128), fp32r inputs.
#
# Slot layout is uniform across images/cores (SPMD: one program). Per image:
#   slot (b, j) for base b in {0,1,2,3} x phase j in {0..nph-1}
#   (+ optional extra slot (0,2) when nph==2).
#   Slot (b,j) objects live in rows [32b,32b+32) of the image's main profile
#   tile (phase mask selects rows per j) and, for j<2, optionally rows
#   [32b,32b+32) of a shared overflow tile (image A owns ovf bases 0,1;
#   image B bases 2,3) whose matmul accumulates (start=False) into the same
#   PSUM blocks.
# PSUM rule respected by construction: each physical PSUM bank only ever
# receives matmuls with one operand base partition (bank 32b <- base 32b);
# mixing bases within a bank hangs the device (found empirically).

import numpy as np

N_IMG, C_CH, H, W, NOBJ, N_CORES = 16, 1, 128, 128, 128, 8
IMGS_PER_CORE = N_IMG // N_CORES  # 2

# layout ladder: (nph, extra_02)
_LADDER = ((2, True), (3, False), (4, False))

_PROG_CACHE = {}


# ---------------------------------------------------------------- host math
def _gaussian_radius_f32(w, h):
    mo = np.float32(0.7)
    one = np.float32(1.0)
    b1 = h + w
    c1 = w * h * (one - mo) / (one + mo)
    sq1 = np.sqrt(b1 * b1 - np.float32(4.0) * np.float32(1.0) * c1)
    r1 = (b1 + sq1) / np.float32(2.0)
    b2 = np.float32(2.0) * (h + w)
    c2 = (one - mo) * w * h
    sq2 = np.sqrt(b2 * b2 - np.float32(4.0) * np.float32(4.0) * c2)
    r2 = (b2 + sq2) / np.float32(2.0)
    a3 = np.float32(4.0) * mo
    b3 = np.float32(-2.0) * mo * (h + w)
    c3 = (mo - one) * w * h
    sq3 = np.sqrt(b3 * b3 - np.float32(4.0) * a3 * c3)
    r3 = (b3 + sq3) / np.float32(2.0)
    return np.minimum(np.minimum(r1, r2), r3)


def _discretize_jax(bboxes):
    """cxi/cyi/ri/valid/sigma replicating the reference called with NUMPY
    inputs (the harness contract): pure-numpy ops stay numpy (IEEE fp32,
    truncating int cast), while everything downstream of jnp.sqrt runs on
    the default jax backend — whose int32 cast ROUNDS on neuron. The mix
    must match the grader's reference bit-for-bit."""
    import jax.numpy as jnp

    b = np.asarray(bboxes)
    # numpy side (exactly the reference's expressions with np inputs)
    x1 = b[..., 0] * W
    y1 = b[..., 1] * H
    x2 = b[..., 2] * W
    y2 = b[..., 3] * H
    bw = x2 - x1
    bh = y2 - y1
    w, h = bw, bh
    mo = 0.7
    b1 = h + w
    c1 = w * h * (1 - mo) / (1 + mo)
    with np.errstate(invalid="ignore"):
        sq1 = jnp.sqrt(b1 ** 2 - 4 * 1.0 * c1)       # -> jax (default dev)
        r1 = (b1 + sq1) / 2
        b2 = 2 * (h + w)
        c2 = (1 - mo) * w * h
        sq2 = jnp.sqrt(b2 ** 2 - 4 * 4.0 * c2)
        r2 = (b2 + sq2) / 2
        a3 = 4 * mo
        b3 = -2 * mo * (h + w)
        c3 = (mo - 1) * w * h
        sq3 = jnp.sqrt(b3 ** 2 - 4 * a3 * c3)
        r3 = (b3 + sq3) / 2
    radius = jnp.minimum(jnp.minimum(r1, r2), r3)
    radius = jnp.maximum(jnp.nan_to_num(radius), 0.0)
    cx = (x1 + x2) * 0.5                              # numpy
    cy = (y1 + y2) * 0.5
    cxi = cx.astype(np.int32)                         # numpy: truncates
    cyi = cy.astype(np.int32)
    ri_j = radius.astype(jnp.int32)                   # backend cast
    valid = (b[..., 4] == 1) & (bw > 0) & (bh > 0)    # numpy
    sigma = (2 * ri_j + 1).astype(jnp.float32) / 6.0  # jax like reference
    return [np.asarray(o) for o in (cxi, cyi, ri_j, valid, sigma)]


def _object_params(bboxes):
    """Per-object ints (via the reference's backend) + gaussian coefs."""
    cxi, cyi, ri, valid, sigma = _discretize_jax(bboxes)

    # device computes s = (x*sqk + bsx)^2 ~= k*(x-cx)^2; mask s <= t.
    k64 = 1.0 / (2.0 * sigma.astype(np.float64) ** 2)
    sqk = np.sqrt(k64).astype(np.float32)
    sqk64 = sqk.astype(np.float64)
    bsx = (-sqk64 * cxi).astype(np.float32)
    bsy = (-sqk64 * cyi).astype(np.float32)
    rif = ri.astype(np.float64)
    t = (sqk64 * sqk64 * (rif * rif + rif + 0.5)).astype(np.float32)
    return cxi, cyi, ri, valid, sqk, bsx, bsy, t


# ------------------------------------------------------------- slot fitting
def _fit_image(cxi, cyi, ri, valid, img_pos, nph, extra_02):
    """Assign one image's objects to slots (b, j). img_pos 0 => ovf bases
    {0,1}; 1 => {2,3}. Ovf rows only usable by phases j<2. Returns list of
    (obj, b, j, pool) or None."""
    x0 = np.maximum(cxi - ri, 0)
    x1 = np.minimum(cxi + ri, W - 1)
    y0 = np.maximum(cyi - ri, 0)
    y1 = np.minimum(cyi + ri, H - 1)
    nobj = len(cxi)
    conf = np.zeros((nobj, nobj), bool)
    for a in range(nobj):
        conf[a] = (
            (np.maximum(x0[a], x0) <= np.minimum(x1[a], x1))
            & (np.maximum(y0[a], y0) <= np.minimum(y1[a], y1))
        )
    np.fill_diagonal(conf, False)
    order = np.argsort(-conf.sum(1), kind="stable")

    allowed = (0, 1) if img_pos == 0 else (2, 3)
    slot_order = [(b, j) for j in range(nph) for b in range(4)]
    if extra_02:
        slot_order.append((0, 2))
    members = {s: [] for s in slot_order}
    main_used = [0, 0, 0, 0]
    ovf_used = [0, 0, 0, 0]
    ovf_cap = [32 if b in allowed else 0 for b in range(4)]
    placements = []
    for o in order:
        if not valid[o]:
            continue
        done = False
        for (b, j) in slot_order:
            ovf_ok = j < 2 and ovf_used[b] < ovf_cap[b]
            if main_used[b] >= 32 and not ovf_ok:
                continue
            if any(conf[o, p] for p in members[(b, j)]):
                continue
            members[(b, j)].append(o)
            if main_used[b] < 32:
                main_used[b] += 1
                pool = "main"
            else:
                ovf_used[b] += 1
                pool = "ovf"
            placements.append((int(o), b, j, pool))
            done = True
            break
        if not done:
            return None
    return placements


# ------------------------------------------------------------- program build
def _build_program(nph, extra_02):
    import concourse.bass as bass
    import concourse.bacc as bacc
    import concourse.tile as tile
    from concourse import mybir

    f32 = mybir.dt.float32
    f32r = mybir.dt.float32  # BISECT: fp32 instead of fp32r
    AF = mybir.ActivationFunctionType
    ALU = mybir.AluOpType

    nph_main = nph + (1 if extra_02 else 0)  # phases materialized in rhs
    ncoef = 2 * (4 + nph_main) + 4 + 2  # per-main (coefs+masks) x2 + ovf

    nc = bacc.Bacc("TRN2", target_bir_lowering=False, debug=False,
                   num_devices=N_CORES)
    xsrow = nc.dram_tensor("xsrow", [1, W], f32, kind="ExternalInput")
    coef = nc.dram_tensor("coef", [128, ncoef], f32, kind="ExternalInput")
    out = nc.dram_tensor("out", [IMGS_PER_CORE, C_CH, H, W], f32,
                         kind="ExternalOutput")

    # coef column bases: MA: 0..(4+nph_main)-1, MB: next, OV: 4 coefs + 2 masks
    cb_ma = 0
    cb_mb = 4 + nph_main
    cb_ov = 2 * (4 + nph_main)

    with tile.TileContext(nc) as tc:
        with tc.tile_pool(name="sb", bufs=1) as sb, \
             tc.tile_pool(name="ps", bufs=1, space="PSUM") as psp:
            # tiny warm-up activations so the ACT function-table load happens
            # immediately instead of after the input DMAs
            wz = sb.tile([128, 1], f32, tag="wz")
            nc.vector.memset(wz[:], 0.0)
            wo = sb.tile([128, 1], f32, tag="wo")
            nc.scalar.activation(wo[:], wz[:], AF.Square)
            nc.scalar.activation(wo[:], wz[:], AF.Exp)

            XS = sb.tile([128, W], f32, tag="xs")
            src = xsrow.ap()
            bcast = bass.AP(tensor=src.tensor, offset=src.offset,
                            ap=[[0, 128]] + [src.ap[-1]])
            nc.sync.dma_start(XS[:], bcast)
            CF = sb.tile([128, ncoef], f32, tag="cf")
            nc.sync.dma_start(CF[:], coef.ap()[:, :])

            # profile sets: (name, coef col base, n rhs phases)
            sets = [("MA", cb_ma, nph_main), ("MB", cb_mb, nph_main),
                    ("OV", cb_ov, 2)]
            gxp = {}   # phase-packed rhs tiles (f32r)
            gy = {}    # gy_all (f32r) per set
            for name, cb, np_ in sets:
                s = sb.tile([128, 2 * W], f32, tag=f"s{name}")
                nc.scalar.activation(s[:, 0:W], XS[:], AF.Square,
                                     bias=CF[:, cb + 1:cb + 2],
                                     scale=CF[:, cb:cb + 1])
                nc.scalar.activation(s[:, W:2 * W], XS[:], AF.Square,
                                     bias=CF[:, cb + 2:cb + 3],
                                     scale=CF[:, cb:cb + 1])
                e = sb.tile([128, 2 * W], f32, tag=f"e{name}")
                nc.scalar.activation(e[:], s[:], AF.Exp, bias=0.0, scale=-1.0)
                gxt = sb.tile([128, W], f32, tag=f"gx{name}")
                nc.vector.scalar_tensor_tensor(
                    gxt[:], s[:, 0:W], CF[:, cb + 3:cb + 4], e[:, 0:W],
                    ALU.is_le, ALU.mult)
                gyall = sb.tile([128, W], f32r, tag=f"gya{name}")
                nc.vector.scalar_tensor_tensor(
                    gyall[:], s[:, W:2 * W], CF[:, cb + 3:cb + 4],
                    e[:, W:2 * W], ALU.is_le, ALU.mult)
                gy[name] = gyall
                gxpt = sb.tile([128, np_ * W], f32r, tag=f"gxp{name}")
                for j in range(np_):
                    nc.vector.tensor_scalar(
                        gxpt[:, j * W:(j + 1) * W], gxt[:],
                        CF[:, cb + 4 + j:cb + 5 + j], None, ALU.mult)
                gxp[name] = gxpt

            for i in range(IMGS_PER_CORE):
                mn = "MA" if i == 0 else "MB"
                ovf_bases = (0, 1) if i == 0 else (2, 3)
                ps = psp.tile([128, 2048], f32, tag=f"ps{i}")
                for b in range(4):
                    base = 32 * b
                    has_ovf = b in ovf_bases
                    # phases 0,1: main (+ ovf accumulate), one N=256 MM each
                    blk = ps[:, 512 * b:512 * b + 2 * W]
                    nc.tensor.matmul(
                        blk, gy[mn][base:base + 32, :],
                        gxp[mn][base:base + 32, 0:2 * W],
                        start=True, stop=not has_ovf,
                        tile_position=(base, 0))
                    if has_ovf:
                        nc.tensor.matmul(
                            blk, gy["OV"][base:base + 32, :],
                            gxp["OV"][base:base + 32, 0:2 * W],
                            start=False, stop=True,
                            tile_position=(base, 0))
                    # phases >= 2 (main rows only)
                    p_hi = (nph_main - 2) if (b == 0 or not extra_02) else (nph - 2)
                    if p_hi > 0:
                        blk2 = ps[:, 512 * b + 2 * W:512 * b + (2 + p_hi) * W]
                        nc.tensor.matmul(
                            blk2, gy[mn][base:base + 32, :],
                            gxp[mn][base:base + 32, 2 * W:(2 + p_hi) * W],
                            start=True, stop=True, tile_position=(base, 0))

                red = sb.tile([128, W], f32, tag=f"red{i}")
                psap = ps[:]
                ap3 = bass.AP(tensor=psap.tensor, offset=psap.offset,
                              ap=[psap.ap[0], [1, 128], [512, 4], [128, nph]])
                nc.vector.tensor_reduce(red[:], ap3,
                                        axis=mybir.AxisListType.XY,
                                        op=ALU.max)
                if extra_02:
                    nc.vector.tensor_max(red[:], red[:],
                                         ps[:, nph * W:(nph + 1) * W])
                nc.sync.dma_start(out.ap()[i, 0], red[:])

    nc.compile()
    return nc, ncoef, (cb_ma, cb_mb, cb_ov), nph_main


# ------------------------------------------------------------------ kernel
def kernel(hm, bboxes):
    from concourse.bass_utils import run_bass_kernel_spmd

    assert hm.shape == (N_IMG, C_CH, H, W)
    assert bboxes.shape == (N_IMG, NOBJ, 5)
    cxi, cyi, ri, valid, sqk, bsx, bsy, t = _object_params(np.asarray(bboxes))

    # choose the smallest uniform layout that fits every image
    placements = None
    for nph, extra_02 in _LADDER:
        ok = []
        for i in range(N_IMG):
            p = _fit_image(cxi[i], cyi[i], ri[i], valid[i],
                           i % IMGS_PER_CORE, nph, extra_02)
            if p is None:
                break
            ok.append(p)
        if len(ok) == N_IMG:
            placements = ok
            break
    if placements is None:
        raise RuntimeError("slot fitting failed for all layouts")

    key = (nph, extra_02)
    if key not in _PROG_CACHE:
        _PROG_CACHE[key] = _build_program(nph, extra_02)
    nc, ncoef, (cb_ma, cb_mb, cb_ov), nph_main = _PROG_CACHE[key]

    xs = np.arange(W, dtype=np.float32)[None, :]
    in_maps = []
    for c in range(N_CORES):
        cf = np.zeros((128, ncoef), np.float32)
        cf[:, cb_ma + 3] = -1.0   # t defaults: mask everything (dummy rows)
        cf[:, cb_mb + 3] = -1.0
        cf[:, cb_ov + 3] = -1.0
        for pos in range(IMGS_PER_CORE):
            img = IMGS_PER_CORE * c + pos
            cbm = cb_ma if pos == 0 else cb_mb
            main_next = [0, 0, 0, 0]
            ovf_next = [0, 0, 0, 0]
            for (o, b, j, pool) in placements[img]:
                if pool == "main":
                    row = 32 * b + main_next[b]
                    main_next[b] += 1
                    col0 = cbm
                else:
                    row = 32 * b + ovf_next[b]
                    ovf_next[b] += 1
                    col0 = cb_ov
                cf[row, col0 + 0] = sqk[img, o]
                cf[row, col0 + 1] = bsx[img, o]
                cf[row, col0 + 2] = bsy[img, o]
                cf[row, col0 + 3] = t[img, o]
                cf[row, col0 + 4 + j] = 1.0
        in_maps.append({"xsrow": xs, "coef": cf})

    res = run_bass_kernel_spmd(nc, in_maps, core_ids=list(range(N_CORES)))
    out = np.concatenate([res.results[c]["out"] for c in range(N_CORES)], 0)
    return out.astype(np.float32)


# revision 12
# speedup vs baseline: 1.6043x; 1.5160x over previous
# CenterNet GT-heatmap kernel for Trainium2 (Bass/Tile), 8-core SPMD.
#
# Math: out[n,0,y,x] = max_o gy[o,y]*gx[o,x], where gx/gy are 1-D Gaussian
# profiles clipped to |d| <= r (separable form of the umich gaussian patch).
# Objects are partitioned per image into "slots" (pairwise rect-disjoint
# sets); within a slot, max == sum, so TensorE computes slots as K=32
# matmuls (sums of rank-1 outer products) into PSUM blocks, and VectorE
# max-reduces the blocks. Phase-packing: one matmul per base covers all
# phases via a phase-masked, concatenated rhs (N = nph*128), fp32r inputs.
#
# Slot layout is uniform across images/cores (SPMD: one program). Per image:
#   slot (b, j) for base b in {0,1,2,3} x phase j in {0..nph-1}
#   (+ optional extra slot (0,2) when nph==2).
#   Slot (b,j) objects live in rows [32b,32b+32) of the image's main profile
#   tile (phase mask selects rows per j) and, for j<2, optionally rows
#   [32b,32b+32) of a shared overflow tile (image A owns ovf bases 0,1;
#   image B bases 2,3) whose matmul accumulates (start=False) into the same
#   PSUM blocks.
# PSUM rule respected by construction: each physical PSUM bank only ever
# receives matmuls with one operand base partition (bank 32b <- base 32b);
# mixing bases within a bank hangs the device (found empirically).

import numpy as np

N_IMG, C_CH, H, W, NOBJ, N_CORES = 16, 1, 128, 128, 128, 8
IMGS_PER_CORE = N_IMG // N_CORES  # 2

# layout ladder: (nph, extra_02)
_LADDER = ((2, True), (3, False), (4, False))

_PROG_CACHE = {}


# ---------------------------------------------------------------- host math
def _gaussian_radius_f32(w, h):
    mo = np.float32(0.7)
    one = np.float32(1.0)
    b1 = h + w
    c1 = w * h * (one - mo) / (one + mo)
    sq1 = np.sqrt(b1 * b1 - np.float32(4.0) * np.float32(1.0) * c1)
    r1 = (b1 + sq1) / np.float32(2.0)
    b2 = np.float32(2.0) * (h + w)
    c2 = (one - mo) * w * h
    sq2 = np.sqrt(b2 * b2 - np.float32(4.0) * np.float32(4.0) * c2)
    r2 = (b2 + sq2) / np.float32(2.0)
    a3 = np.float32(4.0) * mo
    b3 = np.float32(-2.0) * mo * (h + w)
    c3 = (mo - one) * w * h
    sq3 = np.sqrt(b3 * b3 - np.float32(4.0) * a3 * c3)
    r3 = (b3 + sq3) / np.float32(2.0)
    return np.minimum(np.minimum(r1, r2), r3)


def _discretize_jax(bboxes):
    """cxi/cyi/ri/valid/sigma replicating the reference called with NUMPY
    inputs (the harness contract): pure-numpy ops stay numpy (IEEE fp32,
    truncating int cast), while everything downstream of jnp.sqrt runs on
    the default jax backend — whose int32 cast ROUNDS on neuron. The mix
    must match the grader's reference bit-for-bit."""
    import jax.numpy as jnp

    b = np.asarray(bboxes)
    # numpy side (exactly the reference's expressions with np inputs)
    x1 = b[..., 0] * W
    y1 = b[..., 1] * H
    x2 = b[..., 2] * W
    y2 = b[..., 3] * H
    bw = x2 - x1
    bh = y2 - y1
    w, h = bw, bh
    mo = 0.7
    b1 = h + w
    c1 = w * h * (1 - mo) / (1 + mo)
    with np.errstate(invalid="ignore"):
        sq1 = jnp.sqrt(b1 ** 2 - 4 * 1.0 * c1)       # -> jax (default dev)
        r1 = (b1 + sq1) / 2
        b2 = 2 * (h + w)
        c2 = (1 - mo) * w * h
        sq2 = jnp.sqrt(b2 ** 2 - 4 * 4.0 * c2)
        r2 = (b2 + sq2) / 2
        a3 = 4 * mo
        b3 = -2 * mo * (h + w)
        c3 = (mo - 1) * w * h
        sq3 = jnp.sqrt(b3 ** 2 - 4 * a3 * c3)
        r3 = (b3 + sq3) / 2
    radius = jnp.minimum(jnp.minimum(r1, r2), r3)
    radius = jnp.maximum(jnp.nan_to_num(radius), 0.0)
    cx = (x1 + x2) * 0.5                              # numpy
    cy = (y1 + y2) * 0.5
    cxi = cx.astype(np.int32)                         # numpy: truncates
    cyi = cy.astype(np.int32)
    ri_j = radius.astype(jnp.int32)                   # backend cast
    valid = (b[..., 4] == 1) & (bw > 0) & (bh > 0)    # numpy
    sigma = (2 * ri_j + 1).astype(jnp.float32) / 6.0  # jax like reference
    return [np.asarray(o) for o in (cxi, cyi, ri_j, valid, sigma)]


def _object_params(bboxes):
    """Per-object ints (via the reference's backend) + gaussian coefs."""
    cxi, cyi, ri, valid, sigma = _discretize_jax(bboxes)

    # device computes s = (x*sqk + bsx)^2 ~= k*(x-cx)^2; mask s <= t.
    k64 = 1.0 / (2.0 * sigma.astype(np.float64) ** 2)
    sqk = np.sqrt(k64).astype(np.float32)
    sqk64 = sqk.astype(np.float64)
    bsx = (-sqk64 * cxi).astype(np.float32)
    bsy = (-sqk64 * cyi).astype(np.float32)
    rif = ri.astype(np.float64)
    t = (sqk64 * sqk64 * (rif * rif + rif + 0.5)).astype(np.float32)
    return cxi, cyi, ri, valid, sqk, bsx, bsy, t


# ------------------------------------------------------------- slot fitting
def _fit_image(cxi, cyi, ri, valid, img_pos, nph, extra_02):
    """Assign one image's objects to slots (b, j). img_pos 0 => ovf bases
    {0,1}; 1 => {2,3}. Ovf rows only usable by phases j<2. Returns list of
    (obj, b, j, pool) or None."""
    x0 = np.maximum(cxi - ri, 0)
    x1 = np.minimum(cxi + ri, W - 1)
    y0 = np.maximum(cyi - ri, 0)
    y1 = np.minimum(cyi + ri, H - 1)
    nobj = len(cxi)
    conf = np.zeros((nobj, nobj), bool)
    for a in range(nobj):
        conf[a] = (
            (np.maximum(x0[a], x0) <= np.minimum(x1[a], x1))
            & (np.maximum(y0[a], y0) <= np.minimum(y1[a], y1))
        )
    np.fill_diagonal(conf, False)
    order = np.argsort(-conf.sum(1), kind="stable")

    allowed = (0, 1) if img_pos == 0 else (2, 3)
    slot_order = [(b, j) for j in range(nph) for b in range(4)]
    if extra_02:
        slot_order.append((0, 2))
    members = {s: [] for s in slot_order}
    main_used = [0, 0, 0, 0]
    ovf_used = [0, 0, 0, 0]
    ovf_cap = [32 if b in allowed else 0 for b in range(4)]
    placements = []
    for o in order:
        if not valid[o]:
            continue
        done = False
        for (b, j) in slot_order:
            ovf_ok = j < 2 and ovf_used[b] < ovf_cap[b]
            if main_used[b] >= 32 and not ovf_ok:
                continue
            if any(conf[o, p] for p in members[(b, j)]):
                continue
            members[(b, j)].append(o)
            if main_used[b] < 32:
                main_used[b] += 1
                pool = "main"
            else:
                ovf_used[b] += 1
                pool = "ovf"
            placements.append((int(o), b, j, pool))
            done = True
            break
        if not done:
            return None
    return placements


# ------------------------------------------------------------- program build
def _build_program(nph, extra_02):
    import concourse.bass as bass
    import concourse.bacc as bacc
    import concourse.tile as tile
    from concourse import mybir

    f32 = mybir.dt.float32
    f32r = mybir.dt.float32r
    AF = mybir.ActivationFunctionType
    ALU = mybir.AluOpType

    nph_main = nph + (1 if extra_02 else 0)  # phases materialized in rhs
    ncoef = 2 * (4 + nph_main) + 4 + 2  # per-main (coefs+masks) x2 + ovf

    nc = bacc.Bacc("TRN2", target_bir_lowering=False, debug=False,
                   num_devices=N_CORES)
    xsrow = nc.dram_tensor("xsrow", [1, W], f32, kind="ExternalInput")
    coef = nc.dram_tensor("coef", [128, ncoef], f32, kind="ExternalInput")
    out = nc.dram_tensor("out", [IMGS_PER_CORE, C_CH, H, W], f32,
                         kind="ExternalOutput")

    # coef column bases: MA: 0..(4+nph_main)-1, MB: next, OV: 4 coefs + 2 masks
    cb_ma = 0
    cb_mb = 4 + nph_main
    cb_ov = 2 * (4 + nph_main)

    with tile.TileContext(nc) as tc:
        with tc.tile_pool(name="sb", bufs=1) as sb, \
             tc.tile_pool(name="ps", bufs=1, space="PSUM") as psp:
            # tiny warm-up activations so the ACT function-table load happens
            # immediately instead of after the input DMAs
            wz = sb.tile([128, 1], f32, tag="wz")
            nc.vector.memset(wz[:], 0.0)
            wo = sb.tile([128, 1], f32, tag="wo")
            nc.scalar.activation(wo[:], wz[:], AF.Square)
            nc.scalar.activation(wo[:], wz[:], AF.Exp)

            XS = sb.tile([128, W], f32, tag="xs")
            src = xsrow.ap()
            bcast = bass.AP(tensor=src.tensor, offset=src.offset,
                            ap=[[0, 128]] + [src.ap[-1]])
            nc.sync.dma_start(XS[:], bcast)
            CF = sb.tile([128, ncoef], f32, tag="cf")
            nc.sync.dma_start(CF[:], coef.ap()[:, :])

            # profile sets: (name, coef col base, n rhs phases)
            sets = [("MA", cb_ma, nph_main), ("MB", cb_mb, nph_main),
                    ("OV", cb_ov, 2)]
            gxp = {}   # phase-packed rhs tiles (f32r)
            gy = {}    # gy_all (f32r) per set
            for name, cb, np_ in sets:
                s = sb.tile([128, 2 * W], f32, tag=f"s{name}")
                nc.scalar.activation(s[:, 0:W], XS[:], AF.Square,
                                     bias=CF[:, cb + 1:cb + 2],
                                     scale=CF[:, cb:cb + 1])
                nc.scalar.activation(s[:, W:2 * W], XS[:], AF.Square,
                                     bias=CF[:, cb + 2:cb + 3],
                                     scale=CF[:, cb:cb + 1])
                e = sb.tile([128, 2 * W], f32, tag=f"e{name}")
                nc.scalar.activation(e[:], s[:], AF.Exp, bias=0.0, scale=-1.0)
                gxt = sb.tile([128, W], f32, tag=f"gx{name}")
                nc.vector.scalar_tensor_tensor(
                    gxt[:], s[:, 0:W], CF[:, cb + 3:cb + 4], e[:, 0:W],
                    ALU.is_le, ALU.mult)
                gyall = sb.tile([128, W], f32r, tag=f"gya{name}")
                nc.vector.scalar_tensor_tensor(
                    gyall[:], s[:, W:2 * W], CF[:, cb + 3:cb + 4],
                    e[:, W:2 * W], ALU.is_le, ALU.mult)
                gy[name] = gyall
                gxpt = sb.tile([128, np_ * W], f32r, tag=f"gxp{name}")
                for j in range(np_):
                    nc.vector.tensor_scalar(
                        gxpt[:, j * W:(j + 1) * W], gxt[:],
                        CF[:, cb + 4 + j:cb + 5 + j], None, ALU.mult)
                gxp[name] = gxpt

            for i in range(IMGS_PER_CORE):
                mn = "MA" if i == 0 else "MB"
                ovf_bases = (0, 1) if i == 0 else (2, 3)
                ps = psp.tile([128, 2048], f32, tag=f"ps{i}")
                for b in range(4):
                    base = 32 * b
                    has_ovf = b in ovf_bases
                    # phases 0,1: main (+ ovf accumulate), one N=256 MM each
                    blk = ps[:, 512 * b:512 * b + 2 * W]
                    nc.tensor.matmul(
                        blk, gy[mn][base:base + 32, :],
                        gxp[mn][base:base + 32, 0:2 * W],
                        start=True, stop=not has_ovf,
                        tile_position=(base, 0))
                    if has_ovf:
                        nc.tensor.matmul(
                            blk, gy["OV"][base:base + 32, :],
                            gxp["OV"][base:base + 32, 0:2 * W],
                            start=False, stop=True,
                            tile_position=(base, 0))
                    # phases >= 2 (main rows only)
                    p_hi = (nph_main - 2) if (b == 0 or not extra_02) else (nph - 2)
                    if p_hi > 0:
                        blk2 = ps[:, 512 * b + 2 * W:512 * b + (2 + p_hi) * W]
                        nc.tensor.matmul(
                            blk2, gy[mn][base:base + 32, :],
                            gxp[mn][base:base + 32, 2 * W:(2 + p_hi) * W],
                            start=True, stop=True, tile_position=(base, 0))

                red = sb.tile([128, W], f32, tag=f"red{i}")
                psap = ps[:]
                ap3 = bass.AP(tensor=psap.tensor, offset=psap.offset,
                              ap=[psap.ap[0], [1, 128], [512, 4], [128, nph]])
                nc.vector.tensor_reduce(red[:], ap3,
                                        axis=mybir.AxisListType.XY,
                                        op=ALU.max)
                if extra_02:
                    nc.vector.tensor_max(red[:], red[:],
                                         ps[:, nph * W:(nph + 1) * W])
                nc.sync.dma_start(out.ap()[i, 0], red[:])

    nc.compile()
    return nc, ncoef, (cb_ma, cb_mb, cb_ov), nph_main


# ------------------------------------------------------------------ kernel
def kernel(hm, bboxes):
    from concourse.bass_utils import run_bass_kernel_spmd

    assert hm.shape == (N_IMG, C_CH, H, W)
    assert bboxes.shape == (N_IMG, NOBJ, 5)
    cxi, cyi, ri, valid, sqk, bsx, bsy, t = _object_params(np.asarray(bboxes))

    # choose the smallest uniform layout that fits every image
    placements = None
    for nph, extra_02 in _LADDER:
        ok = []
        for i in range(N_IMG):
            p = _fit_image(cxi[i], cyi[i], ri[i], valid[i],
                           i % IMGS_PER_CORE, nph, extra_02)
            if p is None:
                break
            ok.append(p)
        if len(ok) == N_IMG:
            placements = ok
            break
    if placements is None:
        raise RuntimeError("slot fitting failed for all layouts")

    key = (nph, extra_02)
    if key not in _PROG_CACHE:
        _PROG_CACHE[key] = _build_program(nph, extra_02)
    nc, ncoef, (cb_ma, cb_mb, cb_ov), nph_main = _PROG_CACHE[key]

    xs = np.arange(W, dtype=np.float32)[None, :]
    in_maps = []
    for c in range(N_CORES):
        cf = np.zeros((128, ncoef), np.float32)
        cf[:, cb_ma + 3] = -1.0   # t defaults: mask everything (dummy rows)
        cf[:, cb_mb + 3] = -1.0
        cf[:, cb_ov + 3] = -1.0
        for pos in range(IMGS_PER_CORE):
            img = IMGS_PER_CORE * c + pos
            cbm = cb_ma if pos == 0 else cb_mb
            main_next = [0, 0, 0, 0]
            ovf_next = [0, 0, 0, 0]
            for (o, b, j, pool) in placements[img]:
                if pool == "main":
                    row = 32 * b + main_next[b]
                    main_next[b] += 1
                    col0 = cbm
                else:
                    row = 32 * b + ovf_next[b]
                    ovf_next[b] += 1
                    col0 = cb_ov
                cf[row, col0 + 0] = sqk[img, o]
                cf[row, col0 + 1] = bsx[img, o]
                cf[row, col0 + 2] = bsy[img, o]
                cf[row, col0 + 3] = t[img, o]
                cf[row, col0 + 4 + j] = 1.0
        in_maps.append({"xsrow": xs, "coef": cf})

    res = run_bass_kernel_spmd(nc, in_maps, core_ids=list(range(N_CORES)))
    out = np.concatenate([res.results[c]["out"] for c in range(N_CORES)], 0)
    return out.astype(np.float32)


# revision 16
# speedup vs baseline: 1.6146x; 1.0064x over previous
# CenterNet GT-heatmap kernel for Trainium2 (Bass/Tile), 8-core SPMD.
#
# Math: out[n,0,y,x] = max_o gy[o,y]*gx[o,x], where gx/gy are 1-D Gaussian
# profiles clipped to |d| <= r (separable form of the umich gaussian patch).
# Objects are partitioned per image into "slots" (pairwise rect-disjoint
# sets); within a slot, max == sum, so TensorE computes slots as K=32
# matmuls (sums of rank-1 outer products) into PSUM blocks, and VectorE
# max-reduces the blocks. Phase-packing: one matmul per base covers all
# phases via a phase-masked, concatenated rhs (N = nph*128), fp32r inputs.
#
# Slot layout is uniform across images/cores (SPMD: one program). Per image:
#   slot (b, j) for base b in {0,1,2,3} x phase j in {0..nph-1}
#   (+ optional extra slot (0,2) when nph==2).
#   Slot (b,j) objects live in rows [32b,32b+32) of the image's main profile
#   tile (phase mask selects rows per j) and, for j<2, optionally rows
#   [32b,32b+32) of a shared overflow tile (image A owns ovf bases 0,1;
#   image B bases 2,3) whose matmul accumulates (start=False) into the same
#   PSUM blocks.
# PSUM rule respected by construction: each physical PSUM bank only ever
# receives matmuls with one operand base partition (bank 32b <- base 32b);
# mixing bases within a bank hangs the device (found empirically).

import numpy as np

N_IMG, C_CH, H, W, NOBJ, N_CORES = 16, 1, 128, 128, 128, 8
IMGS_PER_CORE = N_IMG // N_CORES  # 2

# layout ladder: (nph, extra_02)
_LADDER = ((2, True), (3, False), (4, False))

_PROG_CACHE = {}


def _discretize_jax(bboxes):
    """cxi/cyi/ri/valid/sigma replicating the reference called with NUMPY
    inputs (the harness contract): pure-numpy ops stay numpy (IEEE fp32,
    truncating int cast), while everything downstream of jnp.sqrt runs on
    the default jax backend — whose int32 cast ROUNDS on neuron. The mix
    must match the grader's reference bit-for-bit."""
    import jax.numpy as jnp

    b = np.asarray(bboxes)
    # numpy side (exactly the reference's expressions with np inputs)
    x1 = b[..., 0] * W
    y1 = b[..., 1] * H
    x2 = b[..., 2] * W
    y2 = b[..., 3] * H
    bw = x2 - x1
    bh = y2 - y1
    w, h = bw, bh
    mo = 0.7
    b1 = h + w
    c1 = w * h * (1 - mo) / (1 + mo)
    with np.errstate(invalid="ignore"):
        sq1 = jnp.sqrt(b1 ** 2 - 4 * 1.0 * c1)       # -> jax (default dev)
        r1 = (b1 + sq1) / 2
        b2 = 2 * (h + w)
        c2 = (1 - mo) * w * h
        sq2 = jnp.sqrt(b2 ** 2 - 4 * 4.0 * c2)
        r2 = (b2 + sq2) / 2
        a3 = 4 * mo
        b3 = -2 * mo * (h + w)
        c3 = (mo - 1) * w * h
        sq3 = jnp.sqrt(b3 ** 2 - 4 * a3 * c3)
        r3 = (b3 + sq3) / 2
    radius = jnp.minimum(jnp.minimum(r1, r2), r3)
    radius = jnp.maximum(jnp.nan_to_num(radius), 0.0)
    cx = (x1 + x2) * 0.5                              # numpy
    cy = (y1 + y2) * 0.5
    cxi = cx.astype(np.int32)                         # numpy: truncates
    cyi = cy.astype(np.int32)
    ri_j = radius.astype(jnp.int32)                   # backend cast
    valid = (b[..., 4] == 1) & (bw > 0) & (bh > 0)    # numpy
    sigma = (2 * ri_j + 1).astype(jnp.float32) / 6.0  # jax like reference
    return [np.asarray(o) for o in (cxi, cyi, ri_j, valid, sigma)]


def _object_params(bboxes):
    """Per-object ints (via the reference's backend) + gaussian coefs."""
    cxi, cyi, ri, valid, sigma = _discretize_jax(bboxes)

    # device computes s = (x*sqk + bsx)^2 ~= k*(x-cx)^2; mask s <= t.
    k64 = 1.0 / (2.0 * sigma.astype(np.float64) ** 2)
    sqk = np.sqrt(k64).astype(np.float32)
    sqk64 = sqk.astype(np.float64)
    bsx = (-sqk64 * cxi).astype(np.float32)
    bsy = (-sqk64 * cyi).astype(np.float32)
    rif = ri.astype(np.float64)
    t = (sqk64 * sqk64 * (rif * rif + rif + 0.5)).astype(np.float32)
    return cxi, cyi, ri, valid, sqk, bsx, bsy, t


# ------------------------------------------------------------- slot fitting
def _fit_image(cxi, cyi, ri, valid, img_pos, nph, extra_02):
    """Assign one image's objects to slots (b, j). img_pos 0 => ovf bases
    {0,1}; 1 => {2,3}. Ovf rows only usable by phases j<2. Returns list of
    (obj, b, j, pool) or None."""
    x0 = np.maximum(cxi - ri, 0)
    x1 = np.minimum(cxi + ri, W - 1)
    y0 = np.maximum(cyi - ri, 0)
    y1 = np.minimum(cyi + ri, H - 1)
    nobj = len(cxi)
    conf = np.zeros((nobj, nobj), bool)
    for a in range(nobj):
        conf[a] = (
            (np.maximum(x0[a], x0) <= np.minimum(x1[a], x1))
            & (np.maximum(y0[a], y0) <= np.minimum(y1[a], y1))
        )
    np.fill_diagonal(conf, False)
    order = np.argsort(-conf.sum(1), kind="stable")

    allowed = (0, 1) if img_pos == 0 else (2, 3)
    slot_order = [(b, j) for j in range(nph) for b in range(4)]
    if extra_02:
        slot_order.append((0, 2))
    members = {s: [] for s in slot_order}
    main_used = [0, 0, 0, 0]
    ovf_used = [0, 0, 0, 0]
    ovf_cap = [32 if b in allowed else 0 for b in range(4)]
    placements = []
    for o in order:
        if not valid[o]:
            continue
        done = False
        for (b, j) in slot_order:
            ovf_ok = j < 2 and ovf_used[b] < ovf_cap[b]
            if main_used[b] >= 32 and not ovf_ok:
                continue
            if any(conf[o, p] for p in members[(b, j)]):
                continue
            members[(b, j)].append(o)
            if main_used[b] < 32:
                main_used[b] += 1
                pool = "main"
            else:
                ovf_used[b] += 1
                pool = "ovf"
            placements.append((int(o), b, j, pool))
            done = True
            break
        if not done:
            return None
    return placements


# ------------------------------------------------------------- program build
def _build_program(nph, extra_02):
    import concourse.bass as bass
    import concourse.bacc as bacc
    import concourse.tile as tile
    from concourse import mybir

    f32 = mybir.dt.float32
    f32r = mybir.dt.float32r
    AF = mybir.ActivationFunctionType
    ALU = mybir.AluOpType

    nph_main = nph + (1 if extra_02 else 0)  # phases materialized in rhs
    ncoef = 2 * (4 + nph_main) + 4 + 2  # per-main (coefs+masks) x2 + ovf

    nc = bacc.Bacc("TRN2", target_bir_lowering=False, debug=False,
                   num_devices=N_CORES)
    coef = nc.dram_tensor("coef", [128, ncoef], f32, kind="ExternalInput")
    out = nc.dram_tensor("out", [IMGS_PER_CORE, C_CH, H, W], f32,
                         kind="ExternalOutput")

    # coef column bases: MA: 0..(4+nph_main)-1, MB: next, OV: 4 coefs + 2 masks
    cb_ma = 0
    cb_mb = 4 + nph_main
    cb_ov = 2 * (4 + nph_main)

    with tile.TileContext(nc) as tc:
        with tc.tile_pool(name="sb", bufs=1) as sb, \
             tc.tile_pool(name="ps", bufs=1, space="PSUM") as psp:
            # tiny warm-up activations so the ACT function-table load happens
            # immediately instead of after the input DMAs
            wz = sb.tile([128, 1], f32, tag="wz")
            nc.vector.memset(wz[:], 0.0)
            wo = sb.tile([128, 1], f32, tag="wo")
            nc.scalar.activation(wo[:], wz[:], AF.Square)
            nc.scalar.activation(wo[:], wz[:], AF.Exp)

            CF = sb.tile([128, ncoef], f32, tag="cf")
            nc.sync.dma_start(CF[:], coef.ap()[:, :])
            # XS[p, x] = x, built on-device (iota is int-only; cast via copy)
            XSI = sb.tile([128, W], mybir.dt.int32, tag="xsi")
            nc.gpsimd.iota(XSI[:], pattern=[[1, W]], base=0,
                           channel_multiplier=0)
            XS = sb.tile([128, W], f32, tag="xs")
            nc.vector.tensor_copy(XS[:], XSI[:])

            # profile sets: (name, coef col base, n rhs phases)
            sets = [("MA", cb_ma, nph_main), ("MB", cb_mb, nph_main),
                    ("OV", cb_ov, 2)]
            gxp = {}   # phase-packed rhs tiles (f32r)
            gy = {}    # gy_all (f32r) per set
            for name, cb, np_ in sets:
                s = sb.tile([128, 2 * W], f32, tag=f"s{name}")
                nc.scalar.activation(s[:, 0:W], XS[:], AF.Square,
                                     bias=CF[:, cb + 1:cb + 2],
                                     scale=CF[:, cb:cb + 1])
                nc.scalar.activation(s[:, W:2 * W], XS[:], AF.Square,
                                     bias=CF[:, cb + 2:cb + 3],
                                     scale=CF[:, cb:cb + 1])
                e = sb.tile([128, 2 * W], f32, tag=f"e{name}")
                nc.scalar.activation(e[:], s[:], AF.Exp, bias=0.0, scale=-1.0)
                gxt = sb.tile([128, W], f32, tag=f"gx{name}")
                nc.vector.scalar_tensor_tensor(
                    gxt[:], s[:, 0:W], CF[:, cb + 3:cb + 4], e[:, 0:W],
                    ALU.is_le, ALU.mult)
                gyall = sb.tile([128, W], f32r, tag=f"gya{name}")
                nc.vector.scalar_tensor_tensor(
                    gyall[:], s[:, W:2 * W], CF[:, cb + 3:cb + 4],
                    e[:, W:2 * W], ALU.is_le, ALU.mult)
                gy[name] = gyall
                gxpt = sb.tile([128, np_ * W], f32r, tag=f"gxp{name}")
                for j in range(np_):
                    nc.vector.tensor_scalar(
                        gxpt[:, j * W:(j + 1) * W], gxt[:],
                        CF[:, cb + 4 + j:cb + 5 + j], None, ALU.mult)
                gxp[name] = gxpt

            for i in range(IMGS_PER_CORE):
                mn = "MA" if i == 0 else "MB"
                ovf_bases = (0, 1) if i == 0 else (2, 3)
                ps = psp.tile([128, 2048], f32, tag=f"ps{i}")
                for b in range(4):
                    base = 32 * b
                    has_ovf = b in ovf_bases
                    # phases 0,1: main (+ ovf accumulate), one N=256 MM each
                    blk = ps[:, 512 * b:512 * b + 2 * W]
                    nc.tensor.matmul(
                        blk, gy[mn][base:base + 32, :],
                        gxp[mn][base:base + 32, 0:2 * W],
                        start=True, stop=not has_ovf,
                        tile_position=(base, 0))
                    if has_ovf:
                        nc.tensor.matmul(
                            blk, gy["OV"][base:base + 32, :],
                            gxp["OV"][base:base + 32, 0:2 * W],
                            start=False, stop=True,
                            tile_position=(base, 0))
                    # phases >= 2 (main rows only)
                    p_hi = (nph_main - 2) if (b == 0 or not extra_02) else (nph - 2)
                    if p_hi > 0:
                        blk2 = ps[:, 512 * b + 2 * W:512 * b + (2 + p_hi) * W]
                        nc.tensor.matmul(
                            blk2, gy[mn][base:base + 32, :],
                            gxp[mn][base:base + 32, 2 * W:(2 + p_hi) * W],
                            start=True, stop=True, tile_position=(base, 0))

                red = sb.tile([128, W], f32, tag=f"red{i}")
                psap = ps[:]
                ap3 = bass.AP(tensor=psap.tensor, offset=psap.offset,
                              ap=[psap.ap[0], [1, 128], [512, 4], [128, nph]])
                nc.vector.tensor_reduce(red[:], ap3,
                                        axis=mybir.AxisListType.XY,
                                        op=ALU.max)
                if extra_02:
                    nc.vector.tensor_max(red[:], red[:],
                                         ps[:, nph * W:(nph + 1) * W])
                # separate DGE queues so the second image's output DMA isn't
                # serialized behind the first one's completion wait on SP
                if i == 0:
                    nc.sync.dma_start(out.ap()[i, 0], red[:])
                else:
                    nc.gpsimd.dma_start(out.ap()[i, 0], red[:])

    nc.compile()
    return nc, ncoef, (cb_ma, cb_mb, cb_ov), nph_main


# ------------------------------------------------------------------ kernel
def kernel(hm, bboxes):
    from concourse.bass_utils import run_bass_kernel_spmd

    assert hm.shape == (N_IMG, C_CH, H, W)
    assert bboxes.shape == (N_IMG, NOBJ, 5)
    cxi, cyi, ri, valid, sqk, bsx, bsy, t = _object_params(np.asarray(bboxes))

    # choose the smallest uniform layout that fits every image
    placements = None
    for nph, extra_02 in _LADDER:
        ok = []
        for i in range(N_IMG):
            p = _fit_image(cxi[i], cyi[i], ri[i], valid[i],
                           i % IMGS_PER_CORE, nph, extra_02)
            if p is None:
                break
            ok.append(p)
        if len(ok) == N_IMG:
            placements = ok
            break
    if placements is None:
        raise RuntimeError("slot fitting failed for all layouts")

    key = (nph, extra_02)
    if key not in _PROG_CACHE:
        _PROG_CACHE[key] = _build_program(nph, extra_02)
    nc, ncoef, (cb_ma, cb_mb, cb_ov), nph_main = _PROG_CACHE[key]

    in_maps = []
    for c in range(N_CORES):
        cf = np.zeros((128, ncoef), np.float32)
        cf[:, cb_ma + 3] = -1.0   # t defaults: mask everything (dummy rows)
        cf[:, cb_mb + 3] = -1.0
        cf[:, cb_ov + 3] = -1.0
        for pos in range(IMGS_PER_CORE):
            img = IMGS_PER_CORE * c + pos
            cbm = cb_ma if pos == 0 else cb_mb
            main_next = [0, 0, 0, 0]
            ovf_next = [0, 0, 0, 0]
            for (o, b, j, pool) in placements[img]:
                if pool == "main":
                    row = 32 * b + main_next[b]
                    main_next[b] += 1
                    col0 = cbm
                else:
                    row = 32 * b + ovf_next[b]
                    ovf_next[b] += 1
                    col0 = cb_ov
                cf[row, col0 + 0] = sqk[img, o]
                cf[row, col0 + 1] = bsx[img, o]
                cf[row, col0 + 2] = bsy[img, o]
                cf[row, col0 + 3] = t[img, o]
                cf[row, col0 + 4 + j] = 1.0
        in_maps.append({"coef": cf})

    res = run_bass_kernel_spmd(nc, in_maps, core_ids=list(range(N_CORES)))
    out = np.concatenate([res.results[c]["out"] for c in range(N_CORES)], 0)
    return out.astype(np.float32)
